# revision 1
# baseline (speedup 1.0000x reference)
"""GCNN (batched SpMM + GEMM + bias + ReLU) Trainium2 kernel.

Per-core work (one graph per NeuronCore, 8 graphs / 8 cores):
  phase 0: y = x @ W           (PE, fp32 in, bf16 out to DRAM)
  phase 1: out = relu(A @ y + b)
    - edges sorted by destination row (host-side index prep)
    - dma_gather y[cols] from DRAM (bf16, 256B rows)
    - one-hot segment matrices built on DVE (iota compare x vals)
    - segment-sum as PE matmuls accumulating into PSUM, 32-row windows
      col-tiled 4-per-PSUM-tile
    - bias + relu on eviction, DMA to DRAM

SPMD: one NEFF for all 8 cores. The chunk->window structure is baked
into the program, so per-window edge capacities are the max over all 8
graphs (rounded up to 128); each graph pads its windows with zero-val
edges.
"""

import sys

if "/opt/trn_rl_repo" not in sys.path:
    sys.path.insert(0, "/opt/trn_rl_repo")

import numpy as np
import ml_dtypes

import concourse.bacc as bacc
import concourse.mybir as mybir
from concourse import tile
from concourse.bass_utils import run_bass_kernel_spmd

BF16 = ml_dtypes.bfloat16

C = 128          # channels (C_IN == C_OUT == 128)
W_WIN = 32       # output rows per window (PSUM col-tile granularity)
WPG = 4          # windows per group (group = 128 output rows)
CALL_TARGET_CHUNKS = 56   # gather-call granularity (chunks)


# ---------------------------------------------------------------- host prep

def _round_up(a, m):
    return (a + m - 1) // m * m


def _prep(edge_rows, edge_cols, edge_vals, n_nodes):
    """Sort each graph's edges by destination row and pack them into a
    window structure shared by all graphs.

    Returns (structure, per_core_arrays).
    """
    Bn, En = edge_rows.shape
    n_win = _round_up(n_nodes, W_WIN) // W_WIN

    counts = np.zeros((Bn, n_win), dtype=np.int64)
    sorted_edges = []
    for g in range(Bn):
        order = np.argsort(edge_rows[g], kind="stable")
        rs = np.asarray(edge_rows[g])[order]
        cs = np.asarray(edge_cols[g])[order]
        vs = np.asarray(edge_vals[g])[order]
        wid = rs // W_WIN
        counts[g] = np.bincount(wid, minlength=n_win)
        sorted_edges.append((rs, cs, vs))

    cap = np.maximum(counts.max(axis=0), 1)
    cap = ((cap + 127) // 128 * 128).astype(np.int64)   # per-window capacity
    win_off = np.zeros(n_win + 1, dtype=np.int64)
    np.cumsum(cap, out=win_off[1:])
    total = int(win_off[-1])                             # padded edge count
    nch_w = (cap // 128).astype(np.int64)                # chunks per window
    chunk_window = np.repeat(np.arange(n_win), nch_w)    # chunk -> window id
    n_chunks = total // 128

    per_core = []
    for g in range(Bn):
        rs, cs, vs = sorted_edges[g]
        cols_p = np.zeros(total, dtype=np.int16)
        rloc_p = np.zeros(total, dtype=np.float32)
        vals_p = np.zeros(total, dtype=np.float32)
        src_off = np.zeros(n_win + 1, dtype=np.int64)
        np.cumsum(counts[g], out=src_off[1:])
        idx_dst = (win_off[:-1].repeat(counts[g])
                   + np.concatenate([np.arange(c) for c in counts[g]]))
        cols_p[idx_dst] = cs
        rloc_p[idx_dst] = rs - (rs // W_WIN) * W_WIN
        vals_p[idx_dst] = vs

        idx16 = np.tile(cols_p.reshape(-1, 16).T, (8, 1))          # [128, total/16]
        # host-built one-hot segment tiles: S[p, j, r] = vals[j*128+p]
        # iff rows_local[j*128+p] == r  (pure placement, no arithmetic)
        S = np.zeros((n_chunks, 128, W_WIN), dtype=BF16)
        jj = np.arange(total) // 128
        pp = np.arange(total) % 128
        S[jj, pp, rloc_p.astype(np.int64)] = vals_p.astype(BF16)
        S = np.ascontiguousarray(S.transpose(1, 0, 2))             # [128, n_chunks, 32]
        per_core.append((idx16, S))

    structure = (n_win, chunk_window, nch_w, total, n_chunks)
    return structure, per_core


def _make_calls(chunk_window, n_win):
    """Split the chunk list into dma_gather calls aligned to group
    boundaries: list of (chunk_lo, chunk_hi)."""
    n_chunks = len(chunk_window)
    group_of_chunk = chunk_window // WPG
    n_groups = int(group_of_chunk[-1]) + 1
    # chunk range per group
    grp_lo = np.searchsorted(group_of_chunk, np.arange(n_groups), side="left")
    grp_hi = np.searchsorted(group_of_chunk, np.arange(n_groups), side="right")
    calls = []
    lo = 0
    cur_lo_chunk = 0
    while lo < n_groups:
        hi = lo
        while hi < n_groups and (grp_hi[hi] - cur_lo_chunk) <= CALL_TARGET_CHUNKS:
            hi += 1
        if hi == lo:           # single huge group: take it anyway
            hi = lo + 1
        calls.append((int(grp_lo[lo]), int(grp_hi[hi - 1])))
        cur_lo_chunk = int(grp_hi[hi - 1])
        lo = hi
    assert calls[-1][1] == n_chunks
    return calls


# ---------------------------------------------------------------- device IR

def build_nc(n_nodes, structure, gather_dtype=mybir.dt.bfloat16):
    n_win, chunk_window, nch_w, total, n_chunks = structure
    n_tiles = _round_up(n_nodes, 128) // 128      # phase-0 row tiles
    n_groups = (n_win + WPG - 1) // WPG
    calls = _make_calls(chunk_window, n_win)
    max_call_chunks = max(hi - lo for lo, hi in calls)

    f32 = mybir.dt.float32
    bf16 = mybir.dt.bfloat16

    nc = bacc.Bacc("TRN2", num_swdge_queues=4)
    xT_d = nc.dram_tensor("xT", [C, n_nodes], f32, kind="ExternalInput")
    W_d = nc.dram_tensor("W", [C, C], f32, kind="ExternalInput")
    bb_d = nc.dram_tensor("b_bcast", [128, C], f32, kind="ExternalInput")
    idx_d = nc.dram_tensor("idx16", [128, total // 16], mybir.dt.int16,
                           kind="ExternalInput")
    s_d = nc.dram_tensor("S", [128, n_chunks * W_WIN], bf16, kind="ExternalInput")
    out_d = nc.dram_tensor("out", [n_nodes, C], f32, kind="ExternalOutput")
    y_d = nc.dram_tensor("y", [n_nodes, C], gather_dtype, kind="Internal")

    # chunk ranges per window for start/stop flags
    win_lo = {}
    win_hi = {}
    for j, w in enumerate(chunk_window):
        win_lo.setdefault(int(w), j)
        win_hi[int(w)] = j

    with tile.TileContext(nc) as tc:
        with (
            tc.tile_pool(name="const", bufs=1) as constp,
            tc.tile_pool(name="p0", bufs=3) as p0pool,
            tc.tile_pool(name="p0ps", bufs=2, space="PSUM") as p0ps,
            tc.tile_pool(name="gat", bufs=3) as gatp,
            tc.tile_pool(name="sm", bufs=3) as smp,
            tc.tile_pool(name="meta", bufs=2) as metap,
            tc.tile_pool(name="acc", bufs=4, space="PSUM") as accp,
            tc.tile_pool(name="ev", bufs=3) as evp,
        ):
            # ---- constants
            w_t = constp.tile([C, C], f32, tag="w")
            nc.sync.dma_start(out=w_t[:], in_=W_d[:])
            bias_t = constp.tile([128, C], f32, tag="bias")
            nc.sync.dma_start(out=bias_t[:], in_=bb_d[:])

            # ---- phase 0: y = x @ W  (tile over rows)
            for t in range(n_tiles):
                rows = min(128, n_nodes - t * 128)
                xt = p0pool.tile([C, 128], f32, tag="xt")
                nc.sync.dma_start(out=xt[:, :rows],
                                  in_=xT_d[:, t * 128:t * 128 + rows])
                yps = p0ps.tile([128, C], f32, tag="yps")
                nc.tensor.matmul(yps[:rows, :], xt[:, :rows], w_t[:],
                                 start=True, stop=True)
                ysb = p0pool.tile([128, C], gather_dtype, tag="ysb")
                nc.vector.tensor_copy(ysb[:rows, :], yps[:rows, :])
                nc.sync.dma_start(out=y_d[t * 128:t * 128 + rows, :],
                                  in_=ysb[:rows, :])

            tc.strict_bb_all_engine_barrier()

            # ---- phase 1: gather + segment matmul
            pending_psum = {}   # group id -> psum tile
            for ci, (c_lo, c_hi) in enumerate(calls):
                nch = c_hi - c_lo
                nidx = nch * 128
                idx_t = metap.tile([128, max_call_chunks * 8], mybir.dt.int16,
                                   tag="idx")
                nc.sync.dma_start(out=idx_t[:, :nch * 8],
                                  in_=idx_d[:, c_lo * 8:c_hi * 8])
                g_t = gatp.tile([128, max_call_chunks, C], gather_dtype, tag="g")
                nc.gpsimd.dma_gather(
                    out_ap=g_t[:, :nch, :],
                    in_ap=y_d[:],
                    idxs_ap=idx_t[:, :nch * 8],
                    num_idxs=nidx,
                    num_idxs_reg=nidx,
                    elem_size=C,
                    single_packet=False,
                    queue_num=ci % 4,
                )
                s_t = smp.tile([128, max_call_chunks, W_WIN], bf16, tag="s")
                nc.sync.dma_start(
                    out=s_t[:, :nch, :].rearrange("p a b -> p (a b)"),
                    in_=s_d[:, c_lo * W_WIN:c_hi * W_WIN])

                for j in range(c_lo, c_hi):
                    w = int(chunk_window[j])
                    grp, v = w // WPG, w % WPG
                    if grp not in pending_psum:
                        pending_psum[grp] = accp.tile([128, C], f32, tag="acc", name=f"acc_{grp}")
                    ps = pending_psum[grp]
                    nc.tensor.matmul(
                        ps[v * W_WIN:(v + 1) * W_WIN, :],
                        s_t[:, j - c_lo, :],
                        g_t[:, j - c_lo, :],
                        start=(j == win_lo[w]),
                        stop=(j == win_hi[w]),
                        tile_position=(0, v * W_WIN),
                        skip_group_check=True,
                    )
                    # group finished -> evict
                    last_win_of_grp = min((grp + 1) * WPG, n_win) - 1
                    if w == last_win_of_grp and j == win_hi[w]:
                        rows = min(128, n_nodes - grp * 128)
                        ot = evp.tile([128, C], f32, tag="ot")
                        nc.vector.tensor_tensor(
                            out=ot[:rows, :], in0=ps[:rows, :],
                            in1=bias_t[:rows, :], op=mybir.AluOpType.add)
                        nc.scalar.activation(
                            out=ot[:rows, :], in_=ot[:rows, :],
                            func=mybir.ActivationFunctionType.Relu)
                        nc.sync.dma_start(
                            out=out_d[grp * 128:grp * 128 + rows, :],
                            in_=ot[:rows, :])
                        del pending_psum[grp]

    nc.finalize()
    return nc


# ---------------------------------------------------------------- entry

def kernel(x, edge_rows, edge_cols, edge_vals, W, b):
    x = np.asarray(x, dtype=np.float32)
    edge_rows = np.asarray(edge_rows)
    edge_cols = np.asarray(edge_cols)
    edge_vals = np.asarray(edge_vals, dtype=np.float32)
    W = np.asarray(W, dtype=np.float32)
    b = np.asarray(b, dtype=np.float32)

    Bn, n_nodes, _ = x.shape
    structure, per_core = _prep(edge_rows, edge_cols, edge_vals, n_nodes)

    nc = build_nc(n_nodes, structure)

    bias_bcast = np.ascontiguousarray(
        np.broadcast_to(b.astype(np.float32), (128, C)))
    in_maps = []
    for g in range(Bn):
        idx16, S = per_core[g]
        in_maps.append({
            "xT": np.ascontiguousarray(x[g].T),
            "W": W,
            "b_bcast": bias_bcast,
            "idx16": idx16,
            "S": S.reshape(128, -1),
        })

    res = run_bass_kernel_spmd(nc, in_maps, list(range(Bn)))
    out = np.stack([np.asarray(r["out"], dtype=np.float32) for r in res.results])
    return out



# revision 2
# speedup vs baseline: 3.2692x; 3.2692x over previous
"""GCNN (batched SpMM + GEMM + bias + ReLU) Trainium2 kernel — dense-stream.

Per-core work (one graph per NeuronCore, 8 graphs / 8 cores):
  out = relu(A @ (x @ W) + b),  A sparse [N, N] with E edges.

Key idea: per-edge gather/scatter DMA is descriptor-throughput-bound on
TRN2 (~6.8 ns/edge through the 4 SWDGE queues), so avoid indexed DMA
entirely.  Materialize A densely on the HOST (N=10000 -> 200 MB bf16 per
graph) and stream it through the PE as the *moving* matmul operand:

  phase 0: y_t = x_t @ W                  (79 tiles, y resident in SBUF bf16)
  main:    out^T[C, dst] = sum_t y_t^T-contraction with A panels
           for each dst-supergroup S (2048 dst cols = 4 PSUM banks):
             for each src tile t: DMA A panel [128 src, 2048 dst] bf16,
               4 matmuls (lhsT = y_t stationary, rhs = A slice, 512 cols)
               accumulating into 4 PSUM tiles
             evict: relu(psum + b) on ACT (fused bias), DMA out^T slice

HBM traffic ~207 MB/core of pure sequential reads (no descriptors), PE
~420 us, wall ~ DMA-bound.  Output is computed transposed [C, N]; the
host transposes it back (free).

SPMD: one NEFF for all 8 cores; per-core data via input tensors.
"""

import sys

if "/opt/trn_rl_repo" not in sys.path:
    sys.path.insert(0, "/opt/trn_rl_repo")

import numpy as np
import ml_dtypes

import concourse.bacc as bacc
import concourse.mybir as mybir
from concourse import tile
from concourse.bass_utils import run_bass_kernel_spmd

BF16 = ml_dtypes.bfloat16

C = 128            # channels (C_IN == C_OUT == 128)
N = 10000          # nodes per graph
SRC_T = 79         # src tiles of 128 (last tile: 16 rows)
NPAD = SRC_T * 128         # 10112
DSTW = 2048        # dst supergroup width (4 PSUM banks x 512)
NSG = 5            # supergroups (5*2048 = 10240 >= N)
DPAD = NSG * DSTW  # 10240


# ---------------------------------------------------------------- host prep

def prep_core_inputs(x, edge_rows, edge_cols, edge_vals, W, b):
    """Build per-core input maps: dense bf16 A panels + transposed x."""
    Bn = x.shape[0]
    b_col = np.ascontiguousarray(b.astype(np.float32).reshape(C, 1))
    in_maps = []
    for g in range(Bn):
        A = np.zeros((NPAD, DPAD), dtype=np.float32)       # [src, dst]
        np.add.at(A, (np.asarray(edge_cols[g]), np.asarray(edge_rows[g])),
                  np.asarray(edge_vals[g]))
        Ab = A.astype(BF16)
        # panel (S, t) contiguous at cols (S*SRC_T + t)*DSTW: [128, NSG*SRC_T*DSTW]
        Ar = np.ascontiguousarray(
            Ab.reshape(SRC_T, 128, NSG, DSTW).transpose(1, 2, 0, 3)
        ).reshape(128, -1)
        in_maps.append({
            "xT": np.ascontiguousarray(x[g].T.astype(np.float32)),
            "W": np.asarray(W, dtype=np.float32),
            "b_col": b_col,
            "A": Ar,
        })
    return in_maps


# ---------------------------------------------------------------- device IR

def build_nc():
    f32 = mybir.dt.float32
    bf16 = mybir.dt.bfloat16

    nc = bacc.Bacc("TRN2")
    xT_d = nc.dram_tensor("xT", [C, N], f32, kind="ExternalInput")
    W_d = nc.dram_tensor("W", [C, C], f32, kind="ExternalInput")
    bcol_d = nc.dram_tensor("b_col", [C, 1], f32, kind="ExternalInput")
    A_d = nc.dram_tensor("A", [128, NSG * SRC_T * DSTW], bf16,
                         kind="ExternalInput")
    outT_d = nc.dram_tensor("outT", [C, DPAD], f32, kind="ExternalOutput")

    with tile.TileContext(nc) as tc:
        with (
            tc.tile_pool(name="const", bufs=1) as constp,
            tc.tile_pool(name="y", bufs=1) as ypool,
            tc.tile_pool(name="p0", bufs=3) as p0pool,
            tc.tile_pool(name="p0ps", bufs=2, space="PSUM") as p0ps,
            tc.tile_pool(name="a", bufs=4) as apool,
            tc.tile_pool(name="acc", bufs=6, space="PSUM") as accp,
            tc.tile_pool(name="ev", bufs=4) as evp,
        ):
            # ---- constants
            w_t = constp.tile([C, C], f32, tag="w")
            nc.sync.dma_start(out=w_t[:], in_=W_d[:])
            bcol = constp.tile([C, 1], f32, tag="bcol")
            nc.sync.dma_start(out=bcol[:], in_=bcol_d[:])

            # ---- phase 0: y = x @ W resident in SBUF (bf16), src-padded zeros
            y_sb = ypool.tile([128, SRC_T * C], bf16, tag="y")
            nc.vector.memset(y_sb[:], 0)
            for t in range(SRC_T):
                rows = min(128, N - t * 128)
                xt = p0pool.tile([C, 128], f32, tag="xt")
                nc.sync.dma_start(out=xt[:, :rows],
                                  in_=xT_d[:, t * 128:t * 128 + rows])
                yps = p0ps.tile([128, C], f32, tag="yps")
                nc.tensor.matmul(yps[:rows, :], xt[:, :rows], w_t[:],
                                 start=True, stop=True)
                nc.vector.tensor_copy(y_sb[:rows, t * C:(t + 1) * C],
                                      yps[:rows, :])

            # ---- main: stream dense A panels, accumulate out^T in PSUM
            for S in range(NSG):
                ps = [accp.tile([128, 512], f32, tag="acc",
                                name=f"acc_{S}_{k}") for k in range(4)]
                for t in range(SRC_T):
                    a_t = apool.tile([128, DSTW], bf16, tag="a")
                    off = (S * SRC_T + t) * DSTW
                    nc.sync.dma_start(out=a_t[:], in_=A_d[:, off:off + DSTW])
                    for k in range(4):
                        nc.tensor.matmul(
                            ps[k][:, :],
                            y_sb[:, t * C:(t + 1) * C],
                            a_t[:, k * 512:(k + 1) * 512],
                            start=(t == 0), stop=(t == SRC_T - 1))
                for k in range(4):
                    ot = evp.tile([128, 512], f32, tag="ot")
                    nc.scalar.activation(
                        out=ot[:], in_=ps[k][:],
                        func=mybir.ActivationFunctionType.Relu,
                        bias=bcol[:])
                    col = S * DSTW + k * 512
                    nc.sync.dma_start(out=outT_d[:, col:col + 512],
                                      in_=ot[:])

    nc.finalize()
    return nc


# ---------------------------------------------------------------- entry

def kernel(x, edge_rows, edge_cols, edge_vals, W, b):
    x = np.asarray(x, dtype=np.float32)
    W = np.asarray(W, dtype=np.float32)
    b = np.asarray(b, dtype=np.float32)

    Bn = x.shape[0]
    in_maps = prep_core_inputs(x, edge_rows, edge_cols, edge_vals, W, b)
    nc = build_nc()
    res = run_bass_kernel_spmd(nc, in_maps, list(range(Bn)))
    out = np.stack([
        np.asarray(r["outT"], dtype=np.float32)[:, :N].T for r in res.results
    ])
    return out


# revision 3
# speedup vs baseline: 4.1509x; 1.2697x over previous
"""GCNN (batched SpMM + GEMM + bias + ReLU) Trainium2 kernel — dense-stream.

Per-core work (one graph per NeuronCore, 8 graphs / 8 cores):
  out = relu(A @ (x @ W) + b),  A sparse [N, N] with E edges.

Key idea: per-edge gather/scatter DMA is descriptor-throughput-bound on
TRN2 (~6.8 ns/edge through the 4 SWDGE queues), so avoid indexed DMA
entirely.  Materialize A densely on the HOST (N=10000 -> 200 MB bf16 per
graph) and stream it through the PE as the *moving* matmul operand:

  phase 0: y_t = x_t @ W                  (79 tiles, y resident in SBUF bf16)
  main:    out^T[C, dst] accumulated in PSUM over src tiles t
           for each dst-supergroup S (<=2048 dst cols = 4 PSUM banks):
             for each src tile t: stream A panel [128 src, SW] bf16
               (4 panels per dma_start), matmuls with lhsT = y_t stationary,
               rhs = A slice (<=512 moving cols), accumulating into PSUM
             evict: relu(psum + b) on ACT (fused per-partition bias),
               DMA out^T slice

HBM traffic ~210 MB/core of pure sequential reads (no descriptors), PE
~420 us, wall ~ DMA-bound.  Output is computed transposed [C, N]; the
host transposes it back (free).

SPMD: one NEFF for all 8 cores; per-core data via input tensors.
"""

import sys

if "/opt/trn_rl_repo" not in sys.path:
    sys.path.insert(0, "/opt/trn_rl_repo")

import numpy as np
import ml_dtypes

import concourse.bacc as bacc
import concourse.mybir as mybir
from concourse import tile
from concourse.bass_utils import run_bass_kernel_spmd

BF16 = ml_dtypes.bfloat16

C = 128            # channels (C_IN == C_OUT == 128)
N = 10000          # nodes per graph
SRC_T = 79         # src tiles of 128 (last tile: 16 rows)
NPAD = SRC_T * 128          # 10112
SW = [2048, 2048, 2048, 2048, 1824]   # dst supergroup widths (sum = 10016)
NSG = len(SW)
DPAD = sum(SW)             # 10016
SG_OFF = np.cumsum([0] + SW).tolist()
# panel (S, t) column offset in the A stream
PAN_OFF = np.cumsum([0] + [SRC_T * w for w in SW]).tolist()
A_COLS = PAN_OFF[-1]       # 79 * 10016
TCHUNK = 4                 # src tiles per dma_start


# ---------------------------------------------------------------- host prep

def prep_core_inputs(x, edge_rows, edge_cols, edge_vals, W, b):
    """Build per-core input maps: dense bf16 A panel stream + transposed x."""
    Bn = x.shape[0]
    b_col = np.ascontiguousarray(b.astype(np.float32).reshape(C, 1))
    in_maps = []
    for g in range(Bn):
        A = np.zeros((NPAD, DPAD), dtype=np.float32)       # [src, dst]
        np.add.at(A, (np.asarray(edge_cols[g]), np.asarray(edge_rows[g])),
                  np.asarray(edge_vals[g]))
        Ab = A.astype(BF16)
        blocks = []
        for S in range(NSG):
            blk = Ab[:, SG_OFF[S]:SG_OFF[S + 1]]           # [NPAD, SW]
            blocks.append(np.ascontiguousarray(
                blk.reshape(SRC_T, 128, SW[S]).transpose(1, 0, 2)
            ).reshape(128, -1))
        Ar = np.ascontiguousarray(np.hstack(blocks))       # [128, A_COLS]
        in_maps.append({
            "xT": np.ascontiguousarray(x[g].T.astype(np.float32)),
            "W": np.asarray(W, dtype=np.float32),
            "b_col": b_col,
            "A": Ar,
        })
    return in_maps


# ---------------------------------------------------------------- device IR

def build_nc():
    f32 = mybir.dt.float32
    bf16 = mybir.dt.bfloat16

    nc = bacc.Bacc("TRN2")
    xT_d = nc.dram_tensor("xT", [C, N], f32, kind="ExternalInput")
    W_d = nc.dram_tensor("W", [C, C], f32, kind="ExternalInput")
    bcol_d = nc.dram_tensor("b_col", [C, 1], f32, kind="ExternalInput")
    A_d = nc.dram_tensor("A", [128, A_COLS], bf16, kind="ExternalInput")
    outT_d = nc.dram_tensor("outT", [C, DPAD], f32, kind="ExternalOutput")

    with tile.TileContext(nc) as tc:
        with (
            tc.tile_pool(name="const", bufs=1) as constp,
            tc.tile_pool(name="y", bufs=SRC_T) as ypool,
            tc.tile_pool(name="p0", bufs=3) as p0pool,
            tc.tile_pool(name="p0ps", bufs=2, space="PSUM") as p0ps,
            tc.tile_pool(name="a", bufs=3) as apool,
            tc.tile_pool(name="acc", bufs=6, space="PSUM") as accp,
            tc.tile_pool(name="ev", bufs=4) as evp,
        ):
            # ---- constants
            w_t = constp.tile([C, C], f32, tag="w")
            nc.sync.dma_start(out=w_t[:], in_=W_d[:])
            bcol = constp.tile([C, 1], f32, tag="bcol")
            nc.sync.dma_start(out=bcol[:], in_=bcol_d[:])

            # ---- phase 0: y = x @ W resident in SBUF (bf16), one tile per t
            ytiles = []
            for t in range(SRC_T):
                rows = min(128, N - t * 128)
                yt = ypool.tile([128, C], bf16, tag="y", name=f"y_{t}")
                if rows < 128:
                    nc.vector.memset(yt[:], 0)
                xt = p0pool.tile([C, 128], f32, tag="xt")
                nc.sync.dma_start(out=xt[:, :rows],
                                  in_=xT_d[:, t * 128:t * 128 + rows])
                yps = p0ps.tile([128, C], f32, tag="yps")
                nc.tensor.matmul(yps[:rows, :], xt[:, :rows], w_t[:],
                                 start=True, stop=True)
                nc.vector.tensor_copy(yt[:rows, :], yps[:rows, :])
                ytiles.append(yt)

            # ---- main: stream dense A panels, accumulate out^T in PSUM
            for S in range(NSG):
                w = SW[S]
                nbank = (w + 511) // 512
                bw = [min(512, w - k * 512) for k in range(nbank)]
                ps = [accp.tile([128, 512], f32, tag="acc",
                                name=f"acc_{S}_{k}") for k in range(nbank)]
                for t0 in range(0, SRC_T, TCHUNK):
                    nt = min(TCHUNK, SRC_T - t0)
                    a_t = apool.tile([128, TCHUNK * 2048], bf16, tag="a")
                    off = PAN_OFF[S] + t0 * w
                    nc.sync.dma_start(out=a_t[:, :nt * w],
                                      in_=A_d[:, off:off + nt * w])
                    for ti in range(nt):
                        t = t0 + ti
                        for k in range(nbank):
                            nc.tensor.matmul(
                                ps[k][:, :bw[k]],
                                ytiles[t][:],
                                a_t[:, ti * w + k * 512:ti * w + k * 512 + bw[k]],
                                start=(t == 0), stop=(t == SRC_T - 1))
                for k in range(nbank):
                    ot = evp.tile([128, 512], f32, tag="ot")
                    nc.scalar.activation(
                        out=ot[:, :bw[k]], in_=ps[k][:, :bw[k]],
                        func=mybir.ActivationFunctionType.Relu,
                        bias=bcol[:])
                    col = SG_OFF[S] + k * 512
                    nc.sync.dma_start(out=outT_d[:, col:col + bw[k]],
                                      in_=ot[:, :bw[k]])

    nc.finalize()
    return nc


# ---------------------------------------------------------------- entry

def kernel(x, edge_rows, edge_cols, edge_vals, W, b):
    x = np.asarray(x, dtype=np.float32)
    W = np.asarray(W, dtype=np.float32)
    b = np.asarray(b, dtype=np.float32)

    Bn = x.shape[0]
    in_maps = prep_core_inputs(x, edge_rows, edge_cols, edge_vals, W, b)
    nc = build_nc()
    res = run_bass_kernel_spmd(nc, in_maps, list(range(Bn)))
    out = np.stack([
        np.asarray(r["outT"], dtype=np.float32)[:, :N].T for r in res.results
    ])
    return out
